# revision 1
# baseline (speedup 1.0000x reference)
"""CompressedSparseAttention Trainium2 Bass kernel.

Shards across 8 NeuronCores as (batch b in {0,1}) x (kv-head group g in
{0..3}).  Each core handles one batch slice with 4 query heads / 1 kv head:
  - streams x[b], transposes tiles on the PE (fp32), projects q (transposed
    layout, fp32r matmuls), k/v (natural layout), applies RoPE,
  - compresses k/v blocks (softmax-weighted, R=16) on the fly,
  - computes the block-score indexer (x-mean trick: scores = x[:nb] @ wiq @
    (wik^T @ mean_t x)), ranks blocks on-device via all-pairs compares, and
    selects the top-64 blocks with a 0/1 selection matmul,
  - runs attention of the 4 heads against the 64 selected compressed blocks
    (logits transposed [s,t], softmax normalization via ones-matmul +
    exp(-ln(sum))), then the row-sharded wo projection.
Host side sums the 4 per-batch partial outputs (row-parallel wo).
"""
import sys

sys.path.insert(0, "/opt/trn_rl_repo")

import math

import numpy as np

import concourse.bacc as bacc
import concourse.mybir as mybir
import concourse.tile as tile
from concourse import bass_utils
from concourse.masks import make_identity

F32 = mybir.dt.float32
F32R = mybir.dt.float32r
AF = mybir.ActivationFunctionType

B, T, DIM = 2, 4096, 2048
H, HKV, HD = 16, 4, 128
NREP, R, TOPK = 4, 16, 64
THETA = 10000.0
NB = T // R          # 256 blocks
KC = DIM // 128      # 16 contraction chunks
TILE_A = 256         # phase-A token tile (2 chunks of 128)
NT_A = T // TILE_A   # 16
TILE_B = 512         # phase-B token tile
NT_B = T // TILE_B   # 8
GQH = H // 4         # 4 q heads per core

_MODULE_CACHE = {}
import os
SKIP_A = os.environ.get("K_SKIP_A") == "1"
SKIP_B = os.environ.get("K_SKIP_B") == "1"



def build_module(dbg=False):
    key = ("nc", dbg)
    if key in _MODULE_CACHE:
        return _MODULE_CACHE[key]
    nc = bacc.Bacc("TRN2", target_bir_lowering=False, debug=False)

    xb_d = nc.dram_tensor("xb", [T, DIM], F32, kind="ExternalInput")
    wq_d = nc.dram_tensor("wq_g", [128, KC, 512], F32, kind="ExternalInput")
    wkvi_d = nc.dram_tensor("wkvi_g", [128, KC, 384], F32, kind="ExternalInput")
    wik_d = nc.dram_tensor("wik_r", [128, KC, 128], F32, kind="ExternalInput")
    wo_d = nc.dram_tensor("wo_g", [128, 4, DIM], F32, kind="ExternalInput")
    cosM_d = nc.dram_tensor("cosM", [128, T], F32, kind="ExternalInput")
    sinM_d = nc.dram_tensor("sinM", [128, T], F32, kind="ExternalInput")
    cosN_d = nc.dram_tensor("cosN", [T, 64], F32, kind="ExternalInput")
    sinN_d = nc.dram_tensor("sinN", [T, 64], F32, kind="ExternalInput")
    psig_d = nc.dram_tensor("psig", [128, 128], F32, kind="ExternalInput")
    bd01_d = nc.dram_tensor("bd01", [128, 8], F32, kind="ExternalInput")
    iota_d = nc.dram_tensor("iota64", [128, 64], F32, kind="ExternalInput")
    cwab_d = nc.dram_tensor("cwab", [128, 256], F32, kind="ExternalInput")
    ones_d = nc.dram_tensor("onesm", [128, 128], F32, kind="ExternalInput")
    tri_d = nc.dram_tensor("tri", [128, 2, 256], F32, kind="ExternalInput")
    y_d = nc.dram_tensor("y", [T, DIM], F32, kind="ExternalOutput")
    if dbg:
        dbg_qrope = nc.dram_tensor("dbg_qrope", [128, GQH, 512], F32, kind="ExternalOutput")
        dbg_ckcv = nc.dram_tensor("dbg_ckcv", [128, 2, 256], F32, kind="ExternalOutput")
        dbg_scol = nc.dram_tensor("dbg_scol", [128, 2], F32, kind="ExternalOutput")
        dbg_rank = nc.dram_tensor("dbg_rank", [128, 2], F32, kind="ExternalOutput")
        dbg_xbar = nc.dram_tensor("dbg_xbar", [128, KC], F32, kind="ExternalOutput")
        dbg_spre = nc.dram_tensor("dbg_spre", [128, 2, 128], F32, kind="ExternalOutput")
        dbg_ckt = nc.dram_tensor("dbg_ckt", [128, 64], F32, kind="ExternalOutput")
        dbg_cvc = nc.dram_tensor("dbg_cvc", [64, 128], F32, kind="ExternalOutput")

    with tile.TileContext(nc) as tc:
        import contextlib

        with contextlib.ExitStack() as ctx:
            const = ctx.enter_context(tc.tile_pool(name="const", bufs=1))
            pers = ctx.enter_context(tc.tile_pool(name="pers", bufs=1))

            ident = const.tile([128, 128], F32)
            make_identity(nc, ident[:])
            psig = const.tile([128, 128], F32R)
            nc.sync.dma_start(psig[:], psig_d[:].bitcast(F32R))
            bd01 = const.tile([128, 8], F32)
            nc.sync.dma_start(bd01[:], bd01_d[:])
            iota64 = const.tile([128, 64], F32)
            nc.sync.dma_start(iota64[:], iota_d[:])
            cwab = const.tile([128, 256], F32)
            nc.sync.dma_start(cwab[:], cwab_d[:])
            ones_f = const.tile([128, 128], F32)
            nc.sync.dma_start(ones_f[:], ones_d[:])
            ones_r = const.tile([128, 128], F32R)
            nc.sync.dma_start(ones_r[:], ones_d[:].bitcast(F32R))
            lnhalf = const.tile([128, 1], F32)
            nc.vector.memset(lnhalf[:], math.log(0.5))
            tri = const.tile([128, 2, 256], F32)
            nc.sync.dma_start(tri[:], tri_d[:])

            # persistent results of phase A
            qrope = pers.tile([128, GQH, T], F32R)      # q^T, roped, per head
            ckcv = pers.tile([128, 2, 256], F32R)       # compressed [k|v], blocks on partitions
            spre = pers.tile([128, 2, 128], F32)        # x[:256] @ wiq
            xacc = pers.tile([128, KC], F32)
            nc.vector.memset(xacc[:], 0.0)

            # ---------------- phase A ----------------
            with contextlib.ExitStack() as actx:
                wpool = actx.enter_context(tc.tile_pool(name="wA", bufs=1))
                wq_sb = wpool.tile([128, KC, 512], F32R)
                nc.sync.dma_start(wq_sb[:], wq_d[:].bitcast(F32R))
                wkvi_sb = wpool.tile([128, KC, 384], F32R)
                nc.sync.dma_start(wkvi_sb[:], wkvi_d[:].bitcast(F32R))

                x_pool = actx.enter_context(tc.tile_pool(name="xA", bufs=2))
                xt_pool = actx.enter_context(tc.tile_pool(name="xtA", bufs=10))
                cos_pool = actx.enter_context(tc.tile_pool(name="cosA", bufs=2))
                qraw_pool = actx.enter_context(tc.tile_pool(name="qrawA", bufs=3))
                tmp_pool = actx.enter_context(tc.tile_pool(name="tmpA", bufs=4))
                kvr_pool = actx.enter_context(tc.tile_pool(name="kvrA", bufs=3))
                sm_pool = actx.enter_context(tc.tile_pool(name="smA", bufs=2))

                tp_psum = actx.enter_context(
                    tc.tile_pool(name="tpP", bufs=3, space="PSUM"))
                proj_psum = actx.enter_context(
                    tc.tile_pool(name="projP", bufs=2, space="PSUM"))
                cmp_psum = actx.enter_context(
                    tc.tile_pool(name="cmpP", bufs=3, space="PSUM"))

                for t in range(0 if SKIP_A else NT_A):
                    t0 = t * TILE_A
                    xn = []
                    for q_ in range(2):
                        xx = x_pool.tile([128, DIM], F32, tag="x")
                        nc.sync.dma_start(
                            xx[:], xb_d[t0 + q_ * 128:t0 + (q_ + 1) * 128, :])
                        xn.append(xx)

                    # transpose x tile: 8 d-pairs -> x~ [128(d), sub(2), 256(t)]
                    xts = []
                    for p in range(8):
                        tp_ps = tp_psum.tile([128, 512], F32, tag="tp")
                        for s in (0, 1):
                            d = 2 * p + s
                            for q_ in (0, 1):
                                nc.tensor.transpose(
                                    tp_ps[:, s * 256 + q_ * 128:
                                          s * 256 + (q_ + 1) * 128],
                                    xn[q_][:, d * 128:(d + 1) * 128],
                                    ident[:])
                        xt = xt_pool.tile([128, 2, 256], F32R, tag="xt")
                        if p % 2 == 0:
                            nc.scalar.activation(
                                xt[:].rearrange("p a b -> p (a b)"), tp_ps[:],
                                AF.Copy)
                        else:
                            nc.vector.tensor_copy(
                                xt[:].rearrange("p a b -> p (a b)"), tp_ps[:])
                        xts.append(xt)

                    cosM_t = cos_pool.tile([128, TILE_A], F32, tag="cm")
                    nc.scalar.dma_start(cosM_t[:], cosM_d[:, t0:t0 + TILE_A])
                    sinM_t = cos_pool.tile([128, TILE_A], F32, tag="sm")
                    nc.scalar.dma_start(sinM_t[:], sinM_d[:, t0:t0 + TILE_A])

                    # q^T projection + rope (transposed layout)
                    for cc in range(GQH):
                        qps = proj_psum.tile([128, 384], F32, tag="proj")
                        for d in range(KC):
                            nc.tensor.matmul(
                                qps[:, 0:TILE_A],
                                wq_sb[:, d, cc * 128:(cc + 1) * 128],
                                xts[d // 2][:, d % 2, :],
                                start=(d == 0), stop=(d == KC - 1))
                        qraw = qraw_pool.tile([128, TILE_A], F32R, tag="qraw")
                        nc.scalar.activation(qraw[:], qps[:, 0:TILE_A], AF.Copy)
                        swp = proj_psum.tile([128, 384], F32, tag="proj")
                        nc.tensor.matmul(swp[:, 0:TILE_A], psig[:], qraw[:],
                                         start=True, stop=True)
                        tmp1 = tmp_pool.tile([128, TILE_A], F32, tag="r1")
                        nc.vector.tensor_mul(tmp1[:], qraw[:].bitcast(F32),
                                             cosM_t[:])
                        tmp2 = tmp_pool.tile([128, TILE_A], F32, tag="r2")
                        nc.vector.tensor_mul(tmp2[:], swp[:, 0:TILE_A],
                                             sinM_t[:])
                        nc.vector.tensor_add(
                            qrope[:, cc, t0:t0 + TILE_A].bitcast(F32R),
                            tmp1[:], tmp2[:])

                    # k/v/(qi) projection, k-rope, compression, x column sums
                    for q_ in range(2):
                        NKV = 384 if t == 0 else 256
                        kps = proj_psum.tile([128, 384], F32, tag="proj")
                        for d in range(KC):
                            nc.tensor.matmul(
                                kps[:, 0:NKV],
                                xts[d // 2][:, d % 2,
                                            q_ * 128:(q_ + 1) * 128],
                                wkvi_sb[:, d, 0:NKV],
                                start=(d == 0), stop=(d == KC - 1))
                        kvr = kvr_pool.tile([128, 256], F32R, tag="kvr")
                        nc.scalar.activation(kvr[:, 128:256], kps[:, 128:256],
                                             AF.Copy)
                        if t == 0:
                            nc.scalar.activation(spre[:, q_, :],
                                                 kps[:, 256:384], AF.Copy)

                        cosN_t = cos_pool.tile([128, 64], F32, tag="cn")
                        nc.scalar.dma_start(
                            cosN_t[:],
                            cosN_d[t0 + q_ * 128:t0 + (q_ + 1) * 128, :])
                        sinN_t = cos_pool.tile([128, 64], F32, tag="sn")
                        nc.scalar.dma_start(
                            sinN_t[:],
                            sinN_d[t0 + q_ * 128:t0 + (q_ + 1) * 128, :])
                        ka = kps[:, 0:128:2]
                        kb = kps[:, 1:128:2]
                        u1 = tmp_pool.tile([128, 64], F32, tag="k1")
                        u2 = tmp_pool.tile([128, 64], F32, tag="k2")
                        nc.vector.tensor_mul(u1[:], ka, cosN_t[:])
                        nc.vector.tensor_mul(u2[:], kb, sinN_t[:])
                        nc.vector.tensor_sub(kvr[:, 0:128:2],
                                             u1[:], u2[:])
                        u3 = tmp_pool.tile([128, 64], F32, tag="k3")
                        u4 = tmp_pool.tile([128, 64], F32, tag="k4")
                        nc.vector.tensor_mul(u3[:], ka, sinN_t[:])
                        nc.vector.tensor_mul(u4[:], kb, cosN_t[:])
                        nc.vector.tensor_add(kvr[:, 1:128:2],
                                             u3[:], u4[:])

                        # x column sums (for the indexer mean)
                        xl_ps = proj_psum.tile([128, 384], F32, tag="proj")
                        for d in range(KC):
                            nc.tensor.matmul(
                                xl_ps[:, d:d + 1],
                                xn[q_][:, d * 128:(d + 1) * 128],
                                ones_f[:, 0:1],
                                start=True, stop=True)
                        nc.vector.tensor_add(xacc[:], xacc[:], xl_ps[:, 0:KC])

                        # compression: blockwise softmax weights + weighted sums
                        sab = sm_pool.tile([128, 2], F32, tag="sab")
                        tk = tmp_pool.tile([128, 128], F32, tag="tk")
                        nc.vector.tensor_mul(tk[:], kvr[:, 0:128].bitcast(F32),
                                             cwab[:, 0:128])
                        nc.vector.reduce_sum(sab[:, 0:1], tk[:],
                                             axis=mybir.AxisListType.X)
                        tk2 = tmp_pool.tile([128, 128], F32, tag="tk2")
                        nc.vector.tensor_mul(tk2[:], kvr[:, 0:128].bitcast(F32),
                                             cwab[:, 128:256])
                        nc.vector.reduce_sum(sab[:, 1:2], tk2[:],
                                             axis=mybir.AxisListType.X)
                        eab = sm_pool.tile([128, 2], F32, tag="eab")
                        nc.scalar.activation(eab[:], sab[:], AF.Exp)
                        wa = sm_pool.tile([128, 8], F32R, tag="wa")
                        nc.vector.tensor_scalar(
                            out=wa[:], in0=bd01[:], scalar1=eab[:, 0:1],
                            scalar2=None, op0=mybir.AluOpType.mult)
                        wb = sm_pool.tile([128, 8], F32R, tag="wb")
                        nc.vector.tensor_scalar(
                            out=wb[:], in0=bd01[:], scalar1=eab[:, 1:2],
                            scalar2=None, op0=mybir.AluOpType.mult)
                        aps = cmp_psum.tile([8, 256], F32, tag="cmp")
                        nc.tensor.matmul(aps[:], wa[:], kvr[:],
                                         start=True, stop=True)
                        bps = cmp_psum.tile([8, 256], F32, tag="cmp")
                        nc.tensor.matmul(bps[:], wb[:], kvr[:],
                                         start=True, stop=True)
                        sps_ = cmp_psum.tile([8, 256], F32, tag="cmp")
                        nc.tensor.matmul(sps_[:, 0:2], bd01[:], eab[:],
                                         start=True, stop=True)
                        sabs = sm_pool.tile([8, 2], F32, tag="sabs")
                        nc.scalar.activation(sabs[:], sps_[:, 0:2], AF.Copy)
                        lns = sm_pool.tile([8, 2], F32, tag="lns")
                        nc.scalar.activation(lns[:], sabs[:], AF.Ln)
                        rs = sm_pool.tile([8, 2], F32, tag="rs")
                        nc.scalar.activation(rs[:], lns[:], AF.Exp,
                                             scale=-1.0, bias=lnhalf[0:8, :])
                        ca = sm_pool.tile([8, 256], F32, tag="ca")
                        nc.vector.tensor_scalar(
                            out=ca[:], in0=aps[:], scalar1=rs[:, 0:1],
                            scalar2=None, op0=mybir.AluOpType.mult)
                        cb = sm_pool.tile([8, 256], F32, tag="cb")
                        nc.vector.tensor_scalar(
                            out=cb[:], in0=bps[:], scalar1=rs[:, 1:2],
                            scalar2=None, op0=mybir.AluOpType.mult)
                        nb0 = 16 * t + 8 * q_
                        ccs = sm_pool.tile([8, 256], F32R, tag="ccs")
                        nc.vector.tensor_add(ccs[:], ca[:], cb[:])
                        nc.sync.dma_start(
                            ckcv[nb0 % 128:nb0 % 128 + 8, nb0 // 128, :],
                            ccs[:])

            # ---------------- top-k selection ----------------
            if True:
                kpool = ctx.enter_context(tc.tile_pool(name="topk", bufs=1))
                kpsum_stack = contextlib.ExitStack()
                kpsum = kpsum_stack.enter_context(
                    tc.tile_pool(name="topkP", bufs=1, space="PSUM"))

                wik_sb = kpool.tile([128, KC, 128], F32)
                nc.sync.dma_start(wik_sb[:], wik_d[:])
                xbar = xacc
                ups = kpsum.tile([128, 128], F32, tag="tka")
                for d in range(KC):
                    nc.tensor.matmul(ups[:, 0:1], wik_sb[:, d, :],
                                     xbar[:, d:d + 1],
                                     start=(d == 0), stop=(d == KC - 1))
                u_sb = kpool.tile([128, 1], F32)
                nc.scalar.activation(u_sb[:], ups[:, 0:1], AF.Copy)
                urow_ps = kpsum.tile([1, 128], F32, tag="tkb")
                nc.tensor.transpose(urow_ps[:], u_sb[:], ident[:])
                urow = kpool.tile([1, 128], F32R)
                nc.scalar.activation(urow[:], urow_ps[:], AF.Copy)
                urep_ps = kpsum.tile([128, 128], F32, tag="tka")
                nc.tensor.matmul(urep_ps[:], ones_r[0:1, :], urow[:],
                                 start=True, stop=True)
                urep = kpool.tile([128, 128], F32)
                nc.scalar.activation(urep[:], urep_ps[:], AF.Copy)

                stmp = kpool.tile([128, 2, 128], F32)
                nc.vector.tensor_mul(
                    stmp[:], spre[:],
                    urep[:].unsqueeze(1).to_broadcast((128, 2, 128)))
                scol = kpool.tile([128, 2], F32)
                nc.vector.reduce_sum(scol[:], stmp[:],
                                     axis=mybir.AxisListType.X)
                srow_sb = kpool.tile([1, 256], F32R)
                for c in range(2):
                    srp = kpsum.tile([1, 128], F32, tag="tkb")
                    nc.tensor.transpose(srp[:], scol[:, c:c + 1], ident[:])
                    nc.scalar.activation(srow_sb[:, c * 128:(c + 1) * 128],
                                         srp[:], AF.Copy)
                srep_ps = kpsum.tile([128, 256], F32, tag="tka")
                nc.tensor.matmul(srep_ps[:], ones_r[0:1, :], srow_sb[:],
                                 start=True, stop=True)
                srep = kpool.tile([128, 256], F32)
                nc.scalar.activation(srep[:], srep_ps[:], AF.Copy)

                # round scol through the same f32r path as srep so the
                # comparison sees identically-rounded values; break ties by
                # block index (stable, like jax.lax.top_k)
                scolr = kpool.tile([128, 2], F32R)
                nc.scalar.activation(scolr[:], scol[:], AF.Copy)
                rank = kpool.tile([128, 2], F32)
                sel = kpool.tile([128, 2, 64], F32R)
                for c in range(2):
                    g = kpool.tile([128, 256], F32)
                    nc.vector.tensor_scalar(
                        out=g[:], in0=srep[:],
                        scalar1=scolr[:, c:c + 1].bitcast(F32),
                        scalar2=None, op0=mybir.AluOpType.is_gt)
                    e = kpool.tile([128, 256], F32)
                    nc.vector.tensor_scalar(
                        out=e[:], in0=srep[:],
                        scalar1=scolr[:, c:c + 1].bitcast(F32),
                        scalar2=None, op0=mybir.AluOpType.is_equal)
                    nc.vector.tensor_mul(e[:], e[:], tri[:, c, :])
                    nc.vector.tensor_add(g[:], g[:], e[:])
                    nc.vector.reduce_sum(rank[:, c:c + 1], g[:],
                                         axis=mybir.AxisListType.X)
                    nc.vector.tensor_scalar(
                        out=sel[:, c, :], in0=iota64[:],
                        scalar1=rank[:, c:c + 1],
                        scalar2=None, op0=mybir.AluOpType.is_equal)

                ckc_ps = kpsum.tile([64, 256], F32, tag="tkc")
                for c in range(2):
                    nc.tensor.matmul(ckc_ps[:], sel[:, c, :], ckcv[:, c, :],
                                     start=(c == 0), stop=(c == 1))
                ckk = kpool.tile([64, 128], F32)
                nc.scalar.activation(ckk[:], ckc_ps[:, 0:128], AF.Copy)
                cvc = kpool.tile([64, 128], F32R)
                nc.scalar.activation(cvc[:], ckc_ps[:, 128:256], AF.Copy)
                ckt_ps = kpsum.tile([128, 64], F32, tag="tka")
                nc.tensor.transpose(ckt_ps[:], ckk[:], ident[0:64, 0:64])
                ckt = kpool.tile([128, 64], F32R)
                nc.scalar.activation(ckt[:], ckt_ps[:], AF.Copy,
                                     scale=1.0 / math.sqrt(HD))
                kpsum_stack.close()
                if dbg:
                    nc.sync.dma_start(dbg_qrope[:], qrope[:, :, 0:512].bitcast(F32))
                    nc.sync.dma_start(dbg_ckcv[:], ckcv[:].bitcast(F32))
                    nc.sync.dma_start(dbg_scol[:], scol[:])
                    nc.sync.dma_start(dbg_rank[:], rank[:])
                    nc.sync.dma_start(dbg_xbar[:], xacc[:])
                    nc.sync.dma_start(dbg_spre[:], spre[:])
                    nc.sync.dma_start(dbg_ckt[:], ckt[:].bitcast(F32))
                    nc.sync.dma_start(dbg_cvc[:], cvc[:].bitcast(F32))

                # ---------------- phase B: attention + wo ----------------
                with contextlib.ExitStack() as bctx:
                    wb_pool = bctx.enter_context(
                        tc.tile_pool(name="wB", bufs=1))
                    wo_sb = wb_pool.tile([128, 4, DIM], F32R)
                    nc.sync.dma_start(wo_sb[:], wo_d[:].bitcast(F32R))
                    p_pool = bctx.enter_context(
                        tc.tile_pool(name="pB", bufs=4))
                    r_pool = bctx.enter_context(
                        tc.tile_pool(name="rB", bufs=4))
                    on_pool = bctx.enter_context(
                        tc.tile_pool(name="onB", bufs=8))
                    y_pool = bctx.enter_context(
                        tc.tile_pool(name="yB", bufs=4))
                    l_psum = bctx.enter_context(
                        tc.tile_pool(name="lP", bufs=2, space="PSUM"))
                    s_psum = bctx.enter_context(
                        tc.tile_pool(name="sP", bufs=2, space="PSUM"))
                    o_psum = bctx.enter_context(
                        tc.tile_pool(name="oP", bufs=2, space="PSUM"))
                    y_psum = bctx.enter_context(
                        tc.tile_pool(name="yP", bufs=2, space="PSUM"))

                    for tb in range(0 if SKIP_B else NT_B):
                        t0 = tb * TILE_B
                        outns = []
                        for h in range(GQH):
                            lps = l_psum.tile([64, TILE_B], F32, tag="l")
                            nc.tensor.matmul(lps[:], ckt[:],
                                             qrope[:, h, t0:t0 + TILE_B],
                                             start=True, stop=True)
                            pp = p_pool.tile([64, TILE_B], F32R, tag="p")
                            nc.scalar.activation(pp[:], lps[:], AF.Exp)
                            sps = s_psum.tile([128, TILE_B], F32, tag="s")
                            nc.tensor.matmul(sps[:], ones_r[0:64, :], pp[:],
                                             start=True, stop=True)
                            ops_ = o_psum.tile([128, TILE_B], F32, tag="o")
                            nc.tensor.matmul(ops_[:], cvc[:], pp[:],
                                             start=True, stop=True)
                            lnS = r_pool.tile([128, TILE_B], F32, tag="ln")
                            nc.scalar.activation(lnS[:], sps[:], AF.Ln)
                            rr = r_pool.tile([128, TILE_B], F32, tag="rr")
                            nc.scalar.activation(rr[:], lnS[:], AF.Exp,
                                                 scale=-1.0)
                            on = on_pool.tile([128, TILE_B], F32R, tag="on")
                            nc.vector.tensor_mul(on[:], ops_[:],
                                                 rr[:])
                            outns.append(on)
                        for tc_ in range(4):
                            for cg in range(4):
                                yps = y_psum.tile([128, 512], F32, tag="y")
                                for h in range(GQH):
                                    nc.tensor.matmul(
                                        yps[:],
                                        outns[h][:,
                                                 tc_ * 128:(tc_ + 1) * 128],
                                        wo_sb[:, h, cg * 512:(cg + 1) * 512],
                                        start=(h == 0), stop=(h == GQH - 1))
                                ys = y_pool.tile([128, 512], F32, tag="ys")
                                if (tc_ + cg) % 2 == 0:
                                    nc.scalar.activation(ys[:], yps[:],
                                                         AF.Copy)
                                else:
                                    nc.vector.tensor_copy(ys[:], yps[:])
                                eng = nc.sync if (tc_ + cg) % 2 == 0 else nc.scalar
                                eng.dma_start(
                                    y_d[t0 + tc_ * 128:t0 + (tc_ + 1) * 128,
                                        cg * 512:(cg + 1) * 512],
                                    ys[:])

    nc.compile()
    _MODULE_CACHE[key] = nc
    return nc


def _host_tables():
    half = HD // 2
    freqs = 1.0 / (THETA ** (np.arange(half, dtype=np.float64) / half))
    ang = np.arange(T, dtype=np.float64)[:, None] * freqs[None, :]
    cosN = np.cos(ang).astype(np.float32)            # [T, 64]
    sinN = np.sin(ang).astype(np.float32)
    cosM = np.empty((128, T), np.float32)
    sinM = np.empty((128, T), np.float32)
    cosM[0::2, :] = cosN.T
    cosM[1::2, :] = cosN.T
    sinM[0::2, :] = -sinN.T
    sinM[1::2, :] = sinN.T
    psig = np.zeros((128, 128), np.float32)
    for i in range(64):
        psig[2 * i, 2 * i + 1] = 1.0
        psig[2 * i + 1, 2 * i] = 1.0
    bd01 = np.zeros((128, 8), np.float32)
    for tt in range(128):
        bd01[tt, tt // 16] = 1.0
    iota64 = np.tile(np.arange(64, dtype=np.float32)[None, :], (128, 1))
    onesm = np.ones((128, 128), np.float32)
    tri = np.zeros((128, 2, 256), np.float32)
    for c in range(2):
        for p in range(128):
            tri[p, c, 0:c * 128 + p] = 1.0
    return dict(cosM=cosM, sinM=sinM, cosN=cosN, sinN=sinN, psig=psig,
                bd01=bd01, iota64=iota64, onesm=onesm, tri=tri)


def _chunk_weights(w):
    # [DIM, N] -> [128, KC, N] with d = c*128 + p
    n = w.shape[1]
    return np.ascontiguousarray(
        w.reshape(KC, 128, n).transpose(1, 0, 2))


def kernel(x, wq, wk, wv, wo, wiq, wik, cwa, cwb):
    x = np.asarray(x, dtype=np.float32)
    tabs = _host_tables()
    cwab = np.concatenate([
        np.tile(np.asarray(cwa, np.float32)[None, :], (128, 1)),
        np.tile(np.asarray(cwb, np.float32)[None, :], (128, 1))], axis=1)

    wiq_c = _chunk_weights(np.asarray(wiq, np.float32))   # [128, KC, 128]
    wik_c = _chunk_weights(np.asarray(wik, np.float32))

    in_maps = []
    for core in range(8):
        b, g = core // 4, core % 4
        wq_g = _chunk_weights(
            np.asarray(wq, np.float32)[:, g * 512:(g + 1) * 512])
        wkv = np.concatenate([
            np.asarray(wk, np.float32)[:, g * 128:(g + 1) * 128],
            np.asarray(wv, np.float32)[:, g * 128:(g + 1) * 128]], axis=1)
        wkvi_g = np.concatenate([_chunk_weights(wkv), wiq_c], axis=2)
        wo_g = np.ascontiguousarray(
            np.asarray(wo, np.float32)[g * 512:(g + 1) * 512, :]
            .reshape(4, 128, DIM).transpose(1, 0, 2))
        in_maps.append({
            "xb": np.ascontiguousarray(x[b]),
            "wq_g": wq_g,
            "wkvi_g": wkvi_g,
            "wik_r": wik_c,
            "wo_g": wo_g,
            "cosM": tabs["cosM"], "sinM": tabs["sinM"],
            "cosN": tabs["cosN"], "sinN": tabs["sinN"],
            "psig": tabs["psig"], "bd01": tabs["bd01"],
            "iota64": tabs["iota64"], "cwab": cwab,
            "onesm": tabs["onesm"], "tri": tabs["tri"],
        })

    nc = build_module()
    res = bass_utils.run_bass_kernel_spmd(
        nc, in_maps, core_ids=list(range(8)), trace=False)

    out = np.zeros((B, T, DIM), np.float32)
    for core in range(8):
        b = core // 4
        out[b] += res.results[core]["y"]
    return out



# revision 11
# speedup vs baseline: 1.7906x; 1.7906x over previous
"""CompressedSparseAttention Trainium2 Bass kernel (optimized).

Sharding: 8 cores = (batch b in {0,1}) x (kv-head group g in {0..3}).
Each core: one batch, 4 q heads, 1 kv head, row-shard of wo; host sums the
4 per-batch partial outputs.

Key layout/engine decisions vs the naive version:
  - x is pre-transposed AND pre-tiled on the host, uploaded as bf16
    ([128, tile, 16*512] = x^T packed per 512-token tile) so the PE never
    transposes activations and DMA runs at full 16KB/partition bursts.
  - all big matmuls run in bf16 (same PE rate as fp32r, half the DMA/SBUF).
  - the Activation engine only ever runs {Exp, Copy} (one act-table set, no
    1.3us table reloads); softmax normalizations use the DVE
    reciprocal_approx_fast custom op instead of Ln/Exp or divide.
  - the indexer mean ki_mean = mean_t(x @ wik) is accumulated on the PE: wik
    rides as a third 128-col block in the k/v projection, and a 1-row
    f32 ones-matmul per token chunk accumulates sum_t ki[t,:] into a
    persistent PSUM column (scores are scale-invariant, so the 1/T is
    dropped).
  - block compression softmax normalizes the block sums (not the weights):
    ca+cb with the 0.5 factor folded into the logit scale (k side) and into
    wo on the host (v side).
  - partial y is written back in bf16, one DMA per 128-token row block.
"""
import sys

sys.path.insert(0, "/opt/trn_rl_repo")

import math

import numpy as np
import ml_dtypes

import concourse.bacc as bacc
import concourse.mybir as mybir
import concourse.tile as tile
from concourse import bass_utils
from concourse.masks import make_identity

F32 = mybir.dt.float32
F32R = mybir.dt.float32r
BF16 = mybir.dt.bfloat16
AF = mybir.ActivationFunctionType

B, T, DIM = 2, 4096, 2048
H, HKV, HD = 16, 4, 128
NREP, R, TOPK = 4, 16, 64
THETA = 10000.0
NB = T // R          # 256 blocks
KC = DIM // 128      # 16 contraction chunks
TILE = 512           # token tile (4 chunks of 128)
NT = T // TILE       # 8
GQH = H // 4         # 4 q heads per core

_MODULE_CACHE = {}


def build_module(dbg=False):
    key = ("nc", dbg)
    if key in _MODULE_CACHE:
        return _MODULE_CACHE[key]
    nc = bacc.Bacc("TRN2", target_bir_lowering=False, debug=False)

    xtp_d = nc.dram_tensor("xtp", [128, NT, KC * TILE], BF16,
                           kind="ExternalInput")
    wq_d = nc.dram_tensor("wq_g", [128, KC, 512], BF16, kind="ExternalInput")
    wkvi_d = nc.dram_tensor("wkvi_g", [128, KC, 384], BF16,
                            kind="ExternalInput")
    wiq_d = nc.dram_tensor("wiq_c", [128, KC, 128], BF16,
                           kind="ExternalInput")
    wo_d = nc.dram_tensor("wo_g", [128, 4, DIM], BF16, kind="ExternalInput")
    cosM_d = nc.dram_tensor("cosM", [128, T], F32, kind="ExternalInput")
    sinM_d = nc.dram_tensor("sinM", [128, T], F32, kind="ExternalInput")
    cosN_d = nc.dram_tensor("cosNp", [128, NB // 8, 64], F32,
                            kind="ExternalInput")
    sinN_d = nc.dram_tensor("sinNp", [128, NB // 8, 64], F32,
                            kind="ExternalInput")
    psig_d = nc.dram_tensor("psig", [128, 128], F32, kind="ExternalInput")
    bd01_d = nc.dram_tensor("bd01", [128, 8], F32, kind="ExternalInput")
    iota_d = nc.dram_tensor("iota64", [128, 64], F32, kind="ExternalInput")
    cwab_d = nc.dram_tensor("cwab2", [128, 2, 128], F32, kind="ExternalInput")
    ones_d = nc.dram_tensor("onesm", [128, 128], F32, kind="ExternalInput")
    tri_d = nc.dram_tensor("tri", [128, 2, 256], F32, kind="ExternalInput")
    y_d = nc.dram_tensor("y", [T, DIM], BF16, kind="ExternalOutput")
    if dbg:
        dbg_qrope = nc.dram_tensor("dbg_qrope", [128, GQH, 512], F32,
                                   kind="ExternalOutput")
        dbg_ckcv = nc.dram_tensor("dbg_ckcv", [128, 2, 256], F32,
                                  kind="ExternalOutput")
        dbg_u = nc.dram_tensor("dbg_u", [128, 1], F32, kind="ExternalOutput")
        dbg_scol = nc.dram_tensor("dbg_scol", [128, 2], F32,
                                  kind="ExternalOutput")
        dbg_rank = nc.dram_tensor("dbg_rank", [128, 2], F32,
                                  kind="ExternalOutput")
        dbg_spre = nc.dram_tensor("dbg_spre", [128, 256], F32,
                                  kind="ExternalOutput")
        dbg_ckt = nc.dram_tensor("dbg_ckt", [128, 64], F32,
                                 kind="ExternalOutput")
        dbg_cvc = nc.dram_tensor("dbg_cvc", [64, 128], F32,
                                 kind="ExternalOutput")

    with tile.TileContext(nc) as tc:
        import contextlib

        with contextlib.ExitStack() as ctx:
            const = ctx.enter_context(tc.tile_pool(name="const", bufs=1))
            pers = ctx.enter_context(tc.tile_pool(name="pers", bufs=1))

            ident = const.tile([128, 128], F32)
            make_identity(nc, ident[:])
            psig = const.tile([128, 128], F32R)
            nc.sync.dma_start(psig[:], psig_d[:].bitcast(F32R))
            bd01 = const.tile([128, 8], F32)
            nc.sync.dma_start(bd01[:], bd01_d[:])
            iota64 = const.tile([128, 64], F32)
            nc.sync.dma_start(iota64[:], iota_d[:])
            cwab = const.tile([128, 2, 128], F32)
            nc.sync.dma_start(cwab[:], cwab_d[:])
            ones_r = const.tile([128, 128], F32R)
            nc.sync.dma_start(ones_r[:], ones_d[:].bitcast(F32R))
            tri = const.tile([128, 2, 256], F32)
            nc.sync.dma_start(tri[:], tri_d[:])
            ones_bf = const.tile([128, 128], BF16)
            nc.vector.memset(ones_bf[:], 1.0)
            onescol = const.tile([128, 1], F32)
            nc.vector.memset(onescol[:], 1.0)
            cosM = const.tile([128, T], F32)
            nc.sync.dma_start(cosM[:], cosM_d[:])
            sinM = const.tile([128, T], F32)
            nc.sync.dma_start(sinM[:], sinM_d[:])
            cosN = const.tile([128, NB // 8, 64], F32)
            nc.sync.dma_start(cosN[:], cosN_d[:])
            sinN = const.tile([128, NB // 8, 64], F32)
            nc.sync.dma_start(sinN[:], sinN_d[:])

            # persistent phase-A results
            qrope = pers.tile([128, GQH, T], BF16)      # q^T, roped, per head
            ckcv = pers.tile([128, 2, 256], F32R)       # compressed [k|v]
            spreT = pers.tile([128, 256], F32)          # qi^T of tokens 0:256

            # persistent PSUM: u accumulator ([:, 0:1]) + sps2 slots
            ups_stack = contextlib.ExitStack()
            ups_pool = ups_stack.enter_context(
                tc.tile_pool(name="upsP", bufs=1, space="PSUM"))
            ups = ups_pool.tile([128, 512], F32)

            # ---------------- phase A ----------------
            with contextlib.ExitStack() as actx:
                wpool = actx.enter_context(tc.tile_pool(name="wA", bufs=1))
                wq_sb = wpool.tile([128, KC, 512], BF16)
                nc.sync.dma_start(wq_sb[:], wq_d[:])
                wkvi_sb = wpool.tile([128, KC, 384], BF16)
                nc.sync.dma_start(wkvi_sb[:], wkvi_d[:])
                wiq_sb = wpool.tile([128, KC, 128], BF16)
                nc.sync.dma_start(wiq_sb[:], wiq_d[:])

                x_pool = actx.enter_context(tc.tile_pool(name="xA", bufs=2))
                qraw_pool = actx.enter_context(
                    tc.tile_pool(name="qrawA", bufs=2))
                tmp_pool = actx.enter_context(tc.tile_pool(name="tmpA",
                                                           bufs=4))
                kvr_pool = actx.enter_context(tc.tile_pool(name="kvrA",
                                                           bufs=4))
                kih_pool = actx.enter_context(tc.tile_pool(name="kihA",
                                                           bufs=4))
                sm_pool = actx.enter_context(tc.tile_pool(name="smA", bufs=3))

                q_psum = actx.enter_context(
                    tc.tile_pool(name="qP", bufs=2, space="PSUM"))
                s_psum = actx.enter_context(
                    tc.tile_pool(name="sP", bufs=1, space="PSUM"))
                k_psum = actx.enter_context(
                    tc.tile_pool(name="kP", bufs=2, space="PSUM"))
                c_psum = actx.enter_context(
                    tc.tile_pool(name="cP", bufs=2, space="PSUM"))

                def xt_load(t):
                    xx = x_pool.tile([128, KC, TILE], BF16, tag="x")
                    nc.sync.dma_start(
                        xx[:].rearrange("p c j -> p (c j)"), xtp_d[:, t, :])
                    return xx

                xt_next = xt_load(0)

                def qproj(t, cc, xt):
                    qps = q_psum.tile([128, TILE], F32, tag="q")
                    for d in range(KC):
                        nc.tensor.matmul(
                            qps[:], wq_sb[:, d, cc * 128:(cc + 1) * 128],
                            xt[:, d, :], start=(d == 0), stop=(d == KC - 1))
                    return qps

                def qrope_fin(t, cc, qps):
                    t0 = t * TILE
                    qraw = qraw_pool.tile([128, TILE], F32R, tag="qraw")
                    nc.scalar.activation(qraw[:], qps[:], AF.Copy)
                    swp = s_psum.tile([128, TILE], F32, tag="s")
                    nc.tensor.matmul(swp[:], psig[:], qraw[:],
                                     start=True, stop=True)
                    tmp1 = tmp_pool.tile([128, TILE], F32, tag="r1")
                    nc.vector.tensor_mul(tmp1[:], qraw[:].bitcast(F32),
                                         cosM[:, t0:t0 + TILE])
                    tmp2 = tmp_pool.tile([128, TILE], F32, tag="r2")
                    nc.vector.tensor_mul(tmp2[:], swp[:],
                                         sinM[:, t0:t0 + TILE])
                    nc.vector.tensor_add(qrope[:, cc, t0:t0 + TILE],
                                         tmp1[:], tmp2[:])

                def kvproj(t, q_, xt):
                    kps = k_psum.tile([128, 384], F32, tag="k")
                    for d in range(KC):
                        nc.tensor.matmul(
                            kps[:], xt[:, d, q_ * 128:(q_ + 1) * 128],
                            wkvi_sb[:, d, :], start=(d == 0),
                            stop=(d == KC - 1))
                    return kps

                def kv_fin(t, q_, kps):
                    # v + ki copies out of PSUM
                    kvr = kvr_pool.tile([128, 256], F32R, tag="kvr")
                    nc.scalar.activation(kvr[:, 128:256], kps[:, 128:256],
                                         AF.Copy)
                    kih = kih_pool.tile([128, 128], F32, tag="kih")
                    nc.scalar.activation(kih[:], kps[:, 256:384], AF.Copy)
                    # k rope (hd pairs are strided on the free axis here)
                    nbc = t * 4 + q_
                    cosN_t = cosN[:, nbc, :]
                    sinN_t = sinN[:, nbc, :]
                    ka = kps[:, 0:128:2]
                    kb = kps[:, 1:128:2]
                    u1 = tmp_pool.tile([128, 64], F32, tag="k1")
                    u2 = tmp_pool.tile([128, 64], F32, tag="k2")
                    nc.vector.tensor_mul(u1[:], ka, cosN_t)
                    nc.vector.tensor_mul(u2[:], kb, sinN_t)
                    nc.vector.tensor_sub(kvr[:, 0:128:2], u1[:], u2[:])
                    u3 = tmp_pool.tile([128, 64], F32, tag="k3")
                    u4 = tmp_pool.tile([128, 64], F32, tag="k4")
                    nc.vector.tensor_mul(u3[:], ka, sinN_t)
                    nc.vector.tensor_mul(u4[:], kb, cosN_t)
                    nc.vector.tensor_add(kvr[:, 1:128:2], u3[:], u4[:])
                    # compression block-softmax scores
                    tkab = tmp_pool.tile([128, 2, 128], F32, tag="tk")
                    nc.vector.tensor_mul(
                        tkab[:], cwab[:],
                        kvr[:, 0:128].bitcast(F32).unsqueeze(1)
                        .to_broadcast((128, 2, 128)))
                    sab = sm_pool.tile([128, 2], F32, tag="sab")
                    nc.vector.reduce_sum(sab[:], tkab[:],
                                         axis=mybir.AxisListType.X)
                    eab = sm_pool.tile([128, 2], F32, tag="eab")
                    nc.scalar.activation(eab[:], sab[:], AF.Exp)
                    wa = sm_pool.tile([128, 8], F32R, tag="wa")
                    nc.vector.tensor_scalar(
                        out=wa[:], in0=bd01[:], scalar1=eab[:, 0:1],
                        scalar2=None, op0=mybir.AluOpType.mult)
                    wb = sm_pool.tile([128, 8], F32R, tag="wb")
                    nc.vector.tensor_scalar(
                        out=wb[:], in0=bd01[:], scalar1=eab[:, 1:2],
                        scalar2=None, op0=mybir.AluOpType.mult)
                    return kvr, kih, wa, wb, eab

                def kv_mm(t, q_, kvr, kih, wa, wb, eab):
                    nbc = t * 4 + q_
                    # per-chunk u partial: sum_t ki[t, :] into its own column
                    nc.tensor.matmul(ups[:, nbc:nbc + 1], kih[:], onescol[:],
                                     start=True, stop=True,
                                     skip_group_check=True)
                    aps = c_psum.tile([8, 512], F32, tag="c")
                    nc.tensor.matmul(aps[:, 0:256], wa[:], kvr[:],
                                     start=True, stop=True)
                    nc.tensor.matmul(aps[:, 256:512], wb[:], kvr[:],
                                     start=True, stop=True)
                    slot = 40 + (nbc % 2) * 2
                    nc.tensor.matmul(ups[0:8, slot:slot + 2], bd01[:], eab[:],
                                     start=True, stop=True,
                                     skip_group_check=True)
                    return aps, slot

                def kv_norm(t, q_, aps, slot):
                    nb0 = (t * 4 + q_) * 8
                    rcp = sm_pool.tile([8, 2], F32, tag="rcp")
                    nc.vector.reciprocal_approx_fast(
                        out=rcp[:], in_=ups[0:8, slot:slot + 2])
                    ca = sm_pool.tile([8, 256], F32, tag="ca")
                    nc.vector.tensor_scalar(
                        out=ca[:], in0=aps[:, 0:256], scalar1=rcp[:, 0:1],
                        scalar2=None, op0=mybir.AluOpType.mult)
                    cb = sm_pool.tile([8, 256], F32, tag="cb")
                    nc.vector.tensor_scalar(
                        out=cb[:], in0=aps[:, 256:512], scalar1=rcp[:, 1:2],
                        scalar2=None, op0=mybir.AluOpType.mult)
                    ccs = sm_pool.tile([8, 256], F32R, tag="ccs")
                    nc.gpsimd.tensor_add(ccs[:], ca[:], cb[:])
                    nc.sync.dma_start(
                        ckcv[nb0 % 128:nb0 % 128 + 8, nb0 // 128, :], ccs[:])

                # software-pipelined schedule: kv chunks first, then q chunks;
                # the previous chunk's dependent matmuls slot in between.
                pend_cmp = None    # (t, q_, aps, slot) awaiting kv_norm
                pend_kv = None     # (t, q_, kps) awaiting kv_fin + kv_mm
                pend_q = None      # (t, cc, qps) awaiting qrope_fin

                for t in range(NT):
                    xt = xt_next
                    if t + 1 < NT:
                        xt_next = xt_load(t + 1)
                    for q_ in range(4):
                        kps = kvproj(t, q_, xt)
                        if pend_kv is not None:
                            pt, pq, pkps = pend_kv
                            parts = kv_fin(pt, pq, pkps)
                            if pend_cmp is not None:
                                kv_norm(*pend_cmp)
                            pend_cmp = (pt, pq) + kv_mm(pt, pq, *parts)
                        pend_kv = (t, q_, kps)
                    if t == 0:
                        # indexer qi^T for the first 256 tokens
                        sps_ = q_psum.tile([128, TILE], F32, tag="q")
                        for d in range(KC):
                            nc.tensor.matmul(
                                sps_[:, 0:256], wiq_sb[:, d, :],
                                xt[:, d, 0:256], start=(d == 0),
                                stop=(d == KC - 1))
                        nc.scalar.activation(spreT[:], sps_[:, 0:256],
                                             AF.Copy)
                    for cc in range(GQH):
                        qps = qproj(t, cc, xt)
                        if pend_kv is not None:
                            pt, pq, pkps = pend_kv
                            parts = kv_fin(pt, pq, pkps)
                            if pend_cmp is not None:
                                kv_norm(*pend_cmp)
                            pend_cmp = (pt, pq) + kv_mm(pt, pq, *parts)
                            pend_kv = None
                        if pend_q is not None:
                            qrope_fin(*pend_q)
                        pend_q = (t, cc, qps)
                if pend_q is not None:
                    qrope_fin(*pend_q)
                    pend_q = None
                if pend_cmp is not None:
                    kv_norm(*pend_cmp)
                    pend_cmp = None

            # ---------------- top-k selection ----------------
            kpool = ctx.enter_context(tc.tile_pool(name="topk", bufs=1))
            u_sb = kpool.tile([128, 1], F32)
            nc.vector.reduce_sum(u_sb[:], ups[:, 0:NB // 8],
                                 axis=mybir.AxisListType.X)
            ups_stack.close()
            kpsum_stack = contextlib.ExitStack()
            kpsum = kpsum_stack.enter_context(
                tc.tile_pool(name="topkP", bufs=2, space="PSUM"))

            scol_ps = kpsum.tile([128, 2], F32, tag="a")
            for c in range(2):
                nc.tensor.matmul(scol_ps[:, c:c + 1],
                                 spreT[:, c * 128:(c + 1) * 128], u_sb[:],
                                 start=True, stop=True)
            scolr = kpool.tile([128, 2], F32R)
            nc.scalar.activation(scolr[:], scol_ps[:], AF.Copy)
            srow_ps = kpsum.tile([1, 256], F32, tag="b")
            nc.tensor.matmul(srow_ps[:], u_sb[:], spreT[:],
                             start=True, stop=True)
            srow_sb = kpool.tile([1, 256], F32R)
            nc.scalar.activation(srow_sb[:], srow_ps[:], AF.Copy)
            srep_ps = kpsum.tile([128, 256], F32, tag="c")
            nc.tensor.matmul(srep_ps[:], ones_r[0:1, :], srow_sb[:],
                             start=True, stop=True)
            srep = kpool.tile([128, 256], F32)
            nc.scalar.activation(srep[:], srep_ps[:], AF.Copy)

            rank = kpool.tile([128, 2], F32)
            sel = kpool.tile([128, 2, 64], F32R)
            for c in range(2):
                g = kpool.tile([128, 256], F32)
                nc.vector.tensor_scalar(
                    out=g[:], in0=srep[:],
                    scalar1=scolr[:, c:c + 1].bitcast(F32),
                    scalar2=None, op0=mybir.AluOpType.is_gt)
                e = kpool.tile([128, 256], F32)
                nc.vector.tensor_scalar(
                    out=e[:], in0=srep[:],
                    scalar1=scolr[:, c:c + 1].bitcast(F32),
                    scalar2=None, op0=mybir.AluOpType.is_equal)
                nc.vector.tensor_mul(e[:], e[:], tri[:, c, :])
                nc.vector.tensor_add(g[:], g[:], e[:])
                nc.vector.reduce_sum(rank[:, c:c + 1], g[:],
                                     axis=mybir.AxisListType.X)
                nc.vector.tensor_scalar(
                    out=sel[:, c, :], in0=iota64[:],
                    scalar1=rank[:, c:c + 1],
                    scalar2=None, op0=mybir.AluOpType.is_equal)

            ckc_ps = kpsum.tile([64, 256], F32, tag="c")
            for c in range(2):
                nc.tensor.matmul(ckc_ps[:], sel[:, c, :], ckcv[:, c, :],
                                 start=(c == 0), stop=(c == 1))
            ckk = kpool.tile([64, 128], F32)
            nc.scalar.activation(ckk[:], ckc_ps[:, 0:128], AF.Copy)
            cvc = kpool.tile([64, 128], BF16)
            nc.scalar.activation(cvc[:], ckc_ps[:, 128:256], AF.Copy)
            ckt_ps = kpsum.tile([128, 64], F32, tag="a")
            nc.tensor.transpose(ckt_ps[:], ckk[:], ident[0:64, 0:64])
            ckt = kpool.tile([128, 64], BF16)
            # 0.5 from w = .5*(wa+wb) is folded here (k side); v side in wo
            nc.scalar.activation(ckt[:], ckt_ps[:], AF.Copy,
                                 scale=0.5 / math.sqrt(HD))
            kpsum_stack.close()
            if dbg:
                dq = kpool.tile([128, GQH, 512], F32)
                nc.vector.tensor_copy(dq[:], qrope[:, :, 0:512])
                nc.sync.dma_start(dbg_qrope[:], dq[:])
                nc.sync.dma_start(dbg_ckcv[:], ckcv[:].bitcast(F32))
                nc.sync.dma_start(dbg_u[:], u_sb[:])
                dsc = kpool.tile([128, 2], F32)
                nc.vector.tensor_copy(dsc[:], scolr[:].bitcast(F32))
                nc.sync.dma_start(dbg_scol[:], dsc[:])
                nc.sync.dma_start(dbg_rank[:], rank[:])
                nc.sync.dma_start(dbg_spre[:], spreT[:])
                dck = kpool.tile([128, 64], F32)
                nc.vector.tensor_copy(dck[:], ckt[:])
                nc.sync.dma_start(dbg_ckt[:], dck[:])
                dcv = kpool.tile([64, 128], F32)
                nc.vector.tensor_copy(dcv[:], cvc[:])
                nc.sync.dma_start(dbg_cvc[:], dcv[:])

            # ---------------- phase B: attention + wo ----------------
            with contextlib.ExitStack() as bctx:
                wb_pool = bctx.enter_context(tc.tile_pool(name="wB", bufs=1))
                wo_sb = wb_pool.tile([128, 4, DIM], BF16)
                nc.sync.dma_start(wo_sb[:], wo_d[:])
                pp_pool = bctx.enter_context(tc.tile_pool(name="pB", bufs=3))
                rr_pool = bctx.enter_context(tc.tile_pool(name="rB", bufs=3))
                on_pool = bctx.enter_context(tc.tile_pool(name="onB", bufs=6))
                y_pool = bctx.enter_context(tc.tile_pool(name="yB", bufs=5))
                l_psum = bctx.enter_context(
                    tc.tile_pool(name="lP", bufs=2, space="PSUM"))
                s_psum = bctx.enter_context(
                    tc.tile_pool(name="ssP", bufs=2, space="PSUM"))
                o_psum = bctx.enter_context(
                    tc.tile_pool(name="oP", bufs=2, space="PSUM"))
                y_psum = bctx.enter_context(
                    tc.tile_pool(name="yP", bufs=2, space="PSUM"))

                for tb in range(NT):
                    t0 = tb * TILE
                    outns = []
                    for h in range(GQH):
                        lps = l_psum.tile([64, TILE], F32, tag="l")
                        nc.tensor.matmul(lps[:], ckt[:],
                                         qrope[:, h, t0:t0 + TILE],
                                         start=True, stop=True)
                        pp = pp_pool.tile([64, TILE], BF16, tag="p")
                        nc.scalar.activation(pp[:], lps[:], AF.Exp)
                        sps = s_psum.tile([128, TILE], F32, tag="s")
                        nc.tensor.matmul(sps[:], ones_bf[0:64, :], pp[:],
                                         start=True, stop=True)
                        ops_ = o_psum.tile([128, TILE], F32, tag="o")
                        nc.tensor.matmul(ops_[:], cvc[:], pp[:],
                                         start=True, stop=True)
                        rr = rr_pool.tile([128, TILE], F32, tag="rr")
                        nc.vector.reciprocal_approx_fast(out=rr[:],
                                                         in_=sps[:])
                        on = on_pool.tile([128, TILE], BF16, tag="on")
                        nc.vector.tensor_mul(on[:], ops_[:], rr[:])
                        outns.append(on)
                    for tc_ in range(4):
                        ys = y_pool.tile([128, DIM], BF16, tag="ys")
                        for cg in range(4):
                            yps = y_psum.tile([128, 512], F32, tag="y")
                            for h in range(GQH):
                                nc.tensor.matmul(
                                    yps[:],
                                    outns[h][:, tc_ * 128:(tc_ + 1) * 128],
                                    wo_sb[:, h, cg * 512:(cg + 1) * 512],
                                    start=(h == 0), stop=(h == GQH - 1))
                            if cg % 2 == 0:
                                nc.scalar.activation(
                                    ys[:, cg * 512:(cg + 1) * 512], yps[:],
                                    AF.Copy)
                            else:
                                nc.vector.tensor_copy(
                                    ys[:, cg * 512:(cg + 1) * 512], yps[:])
                        nc.sync.dma_start(
                            y_d[t0 + tc_ * 128:t0 + (tc_ + 1) * 128, :],
                            ys[:])

    nc.compile()
    _MODULE_CACHE[key] = nc
    return nc


def _host_tables():
    half = HD // 2
    freqs = 1.0 / (THETA ** (np.arange(half, dtype=np.float64) / half))
    ang = np.arange(T, dtype=np.float64)[:, None] * freqs[None, :]
    cosN = np.cos(ang).astype(np.float32)            # [T, 64]
    sinN = np.sin(ang).astype(np.float32)
    cosM = np.empty((128, T), np.float32)
    sinM = np.empty((128, T), np.float32)
    cosM[0::2, :] = cosN.T
    cosM[1::2, :] = cosN.T
    sinM[0::2, :] = -sinN.T
    sinM[1::2, :] = sinN.T
    # k-rope tables packed per 128-token chunk: [128, chunk, 64]
    cosNp = np.ascontiguousarray(
        cosN.reshape(NB // 8, 128, 64).transpose(1, 0, 2))
    sinNp = np.ascontiguousarray(
        sinN.reshape(NB // 8, 128, 64).transpose(1, 0, 2))
    psig = np.zeros((128, 128), np.float32)
    for i in range(64):
        psig[2 * i, 2 * i + 1] = 1.0
        psig[2 * i + 1, 2 * i] = 1.0
    bd01 = np.zeros((128, 8), np.float32)
    for tt in range(128):
        bd01[tt, tt // 16] = 1.0
    iota64 = np.tile(np.arange(64, dtype=np.float32)[None, :], (128, 1))
    onesm = np.ones((128, 128), np.float32)
    tri = np.zeros((128, 2, 256), np.float32)
    for c in range(2):
        for p in range(128):
            tri[p, c, 0:c * 128 + p] = 1.0
    return dict(cosM=cosM, sinM=sinM, cosNp=cosNp, sinNp=sinNp, psig=psig,
                bd01=bd01, iota64=iota64, onesm=onesm, tri=tri)


def _chunk_weights(w):
    # [DIM, N] -> [128, KC, N] with d = c*128 + p
    n = w.shape[1]
    return np.ascontiguousarray(w.reshape(KC, 128, n).transpose(1, 0, 2))


def _bf16(a):
    return np.ascontiguousarray(a).astype(ml_dtypes.bfloat16)


def kernel(x, wq, wk, wv, wo, wiq, wik, cwa, cwb):
    x = np.asarray(x, dtype=np.float32)
    tabs = _host_tables()
    cwab2 = np.stack([
        np.tile(np.asarray(cwa, np.float32)[None, :], (128, 1)),
        np.tile(np.asarray(cwb, np.float32)[None, :], (128, 1))], axis=1)

    wiq_c = _bf16(_chunk_weights(np.asarray(wiq, np.float32)))
    wik_cn = np.asarray(wik, np.float32)

    # x^T packed per 512-token tile: [128, NT, KC*TILE]
    xtp = {}
    for b in range(B):
        xt = x[b].T.reshape(KC, 128, NT, TILE).transpose(1, 2, 0, 3)
        xtp[b] = _bf16(xt.reshape(128, NT, KC * TILE))

    in_maps = []
    for core in range(8):
        b, g = core // 4, core % 4
        wq_g = _bf16(_chunk_weights(
            np.asarray(wq, np.float32)[:, g * 512:(g + 1) * 512]))
        wkvi = np.concatenate([
            np.asarray(wk, np.float32)[:, g * 128:(g + 1) * 128],
            np.asarray(wv, np.float32)[:, g * 128:(g + 1) * 128],
            wik_cn], axis=1)
        wkvi_g = _bf16(_chunk_weights(wkvi))
        wo_g = _bf16(
            (np.asarray(wo, np.float32)[g * 512:(g + 1) * 512, :] * 0.5)
            .reshape(4, 128, DIM).transpose(1, 0, 2))
        in_maps.append({
            "xtp": xtp[b],
            "wq_g": wq_g,
            "wkvi_g": wkvi_g,
            "wiq_c": wiq_c,
            "wo_g": wo_g,
            "cosM": tabs["cosM"], "sinM": tabs["sinM"],
            "cosNp": tabs["cosNp"], "sinNp": tabs["sinNp"],
            "psig": tabs["psig"], "bd01": tabs["bd01"],
            "iota64": tabs["iota64"], "cwab2": cwab2,
            "onesm": tabs["onesm"], "tri": tabs["tri"],
        })

    nc = build_module()
    res = bass_utils.run_bass_kernel_spmd(
        nc, in_maps, core_ids=list(range(8)), trace=False)

    out = np.zeros((B, T, DIM), np.float32)
    for core in range(8):
        b = core // 4
        out[b] += res.results[core]["y"].astype(np.float32)
    return out
